# revision 23
# baseline (speedup 1.0000x reference)
"""BoxPromptFilter Trainium2 kernel.

Per (image, category) group of 1024 boxes: a box is kept iff it is valid and
the total area of other valid boxes fully contained in it is <= 0.8 * its own
area; kept boxes are compacted to the front (stable), the rest zeroed; if
nothing is kept the group passes through unchanged with count 0.

Key algorithmic fact (verified against the reference): the reference's sort
by area only permutes the summation terms - the keep decision per box is a
pure function of the containment-weighted sum in any order, so no sort is
needed on device.

Layout per core (16 groups): partition = container box k (8 tiles of 128),
free = containee box j (1024). Per k-tile:
  s1 = Sign(x1_j - x1_k), s2 = Sign(y1_j - y1_k), s3 = Sign(x2_k - x2_j)  [ACT]
  s4 = (y2_j <= y2_k)                                                     [DVE]
  m  = min(s1, s2, s3, s4)    (1 iff contained; diagonal -> 0 via Sign(0)=0)
  S_k = sum_j max(m,0) * w_j  (one fused scalar_tensor_tensor with accum)
Then per group: keep -> prefix-sum (PE triangular matmuls) -> permutation
tau -> indirect scatter of masked rows into the output.
"""

import numpy as np

N_CORES = 8
G = 16          # groups per core
M = 1024        # boxes per group
NT = 8          # k-tiles per group (M / 128)
P = 128

_CACHE = {}


def _build_program():
    if "nc" in _CACHE:
        return _CACHE["nc"]

    from contextlib import ExitStack
    import json as _json
    import concourse.bass as bass
    import concourse.tile as tile
    import concourse.bass_utils as _bu
    import concourse.bass2jax as _b2j
    from concourse import mybir

    # --- Workaround: this walrus build supports at most ONE sync-wait per
    # instruction. Split multi-wait instructions by inserting NoOps that each
    # carry one wait, at the BIR-JSON level, for every compile path.
    if not getattr(_bu, "_wait_split_installed", False):
        _orig_compile_bir_kernel = _bu.compile_bir_kernel

        def _split_multi_waits(bir_bytes):
            m = _json.loads(bir_bytes)
            n = 0
            for f in m["functions"]:
                for b in f["blocks"]:
                    new = []
                    for ins in b["instructions"]:
                        si = ins.get("sync_info")
                        waits = (si or {}).get("on_wait") or []
                        if len(waits) > 1:
                            for w in waits[:-1]:
                                n += 1
                                new.append({
                                    "debug": ins.get("debug", 0),
                                    "engine": ins["engine"], "ins": [],
                                    "name": f"I-WSPLIT{n}", "opcode": "NoOp",
                                    "outs": [],
                                    "sync_info": {"on_update": [],
                                                  "on_wait": [w]}})
                            si["on_wait"] = [waits[-1]]
                        new.append(ins)
                    b["instructions"] = new
            return _json.dumps(m).encode()

        def _patched_compile_bir_kernel(bir_json, tmpdir, neff_name="file.neff",
                                        **kw):
            return _orig_compile_bir_kernel(_split_multi_waits(bir_json),
                                            tmpdir, neff_name, **kw)

        _bu.compile_bir_kernel = _patched_compile_bir_kernel
        _b2j.compile_bir_kernel = _patched_compile_bir_kernel
        _bu._wait_split_installed = True

    f32 = mybir.dt.float32
    bf16 = mybir.dt.bfloat16
    i32 = mybir.dt.int32
    Op = mybir.AluOpType
    Act = mybir.ActivationFunctionType

    def fap(t, off, dims):
        # AP over tile t's free space: keep partition dim, custom free dims.
        return bass.AP(tensor=t.tensor, offset=t.offset + off,
                       ap=[t.ap[0]] + dims)

    def bcast_ap(src_row, free_dims):
        # Partition-broadcast read of a single row (PSUM or DRAM or SBUF).
        return bass.AP(tensor=src_row.tensor, offset=src_row.offset,
                       ap=[[0, P]] + free_dims)

    nc = bass.Bass()
    boxesT = nc.dram_tensor("boxesT", [G, 5, M], f32, kind="ExternalInput")
    nf = nc.dram_tensor("nf", [1, G], f32, kind="ExternalInput")
    out_boxes = nc.dram_tensor("out_boxes", [G * M, 5], f32,
                               kind="ExternalOutput")
    out_cnt = nc.dram_tensor("out_cnt", [1, G], i32, kind="ExternalOutput")
    dbg = nc.dram_tensor("dbg", [P, G * NT], f32, kind="ExternalOutput")
    wT_dram = nc.dram_tensor("wT_scratch", [P, P], f32, kind="Internal")
    ck_dram = nc.dram_tensor("ck_scratch", [G, NT + 1], f32, kind="Internal")

    bt = boxesT[:, :, :]
    wtd = wT_dram[:, :]
    ckd = ck_dram[:, :]

    with tile.TileContext(nc) as tc:
        with ExitStack() as ctx:
            consts = ctx.enter_context(tc.tile_pool(name="consts", bufs=1))
            glob = ctx.enter_context(tc.tile_pool(name="glob", bufs=1))
            bcp = ctx.enter_context(tc.tile_pool(name="bcp", bufs=2))
            work = ctx.enter_context(tc.tile_pool(name="work", bufs=4))
            sml = ctx.enter_context(tc.tile_pool(name="sml", bufs=4))
            psum_w = ctx.enter_context(
                tc.tile_pool(name="psum_w", bufs=1, space="PSUM"))
            psum_g = ctx.enter_context(
                tc.tile_pool(name="psum_g", bufs=2, space="PSUM"))

            # ---------------- global prep ----------------
            # coords_sb[p, (g c t)] = boxesT[g, c, t*128 + p]
            coords_sb = consts.tile([P, G, 5, NT], f32)
            nc.sync.dma_start(
                out=coords_sb[:, :, :, :],
                in_=bass.AP(tensor=bt.tensor, offset=0,
                            ap=[[1, P], [5 * M, G], [M, 5], [P, NT]]))
            # negated x1,y1 columns (ACT Sign biases)
            neg01 = consts.tile([P, G, 2, NT], f32)
            nc.vector.tensor_scalar(out=neg01[:, :, :, :],
                                    in0=coords_sb[:, :, 0:2, :],
                                    scalar1=-1.0, scalar2=None, op0=Op.mult)
            # n broadcast [128, G]
            n_bc = consts.tile([P, G], f32)
            nc.sync.dma_start(out=n_bc[:, :],
                              in_=bcast_ap(nf[:1, :], [[1, G]]))
            # iota over k within tile: val = t*128 + p  -> [128, NT]
            iota8 = consts.tile([P, NT], f32)
            nc.gpsimd.iota(out=iota8[:, :], pattern=[[P, NT]], base=0,
                           channel_multiplier=1,
                           allow_small_or_imprecise_dtypes=True)
            # L[p', p] = 1 if p' < p ; identity
            ici = consts.tile([P, P], i32)
            iri = consts.tile([P, P], i32)
            nc.gpsimd.iota(out=ici[:, :], pattern=[[0, P]], base=0,
                           channel_multiplier=1)
            nc.gpsimd.iota(out=iri[:, :], pattern=[[1, P]], base=0,
                           channel_multiplier=0)
            Lmat = consts.tile([P, P], f32)
            nc.vector.tensor_tensor(out=Lmat[:, :], in0=ici[:, :],
                                    in1=iri[:, :], op=Op.is_lt)
            ident = consts.tile([P, P], f32)
            nc.vector.tensor_tensor(out=ident[:, :], in0=ici[:, :],
                                    in1=iri[:, :], op=Op.is_equal)
            ones_col = consts.tile([P, 1], f32)
            nc.vector.memset(ones_col[:, :], 1.0)

            # valid[p,(g,t)] = iota8 < n_g ; area; w = area*valid; thr
            valid = glob.tile([P, G, NT], f32)
            nc.vector.tensor_tensor(
                out=valid[:, :, :],
                in0=fap(iota8, 0, [[0, G], [1, NT]]),
                in1=fap(n_bc, 0, [[1, G], [0, NT]]), op=Op.is_lt)
            dx = glob.tile([P, G, NT], f32)
            nc.vector.tensor_tensor(out=dx[:, :, :],
                                    in0=coords_sb[:, :, 2, :],
                                    in1=coords_sb[:, :, 0, :], op=Op.subtract)
            dy = glob.tile([P, G, NT], f32)
            nc.vector.tensor_tensor(out=dy[:, :, :],
                                    in0=coords_sb[:, :, 3, :],
                                    in1=coords_sb[:, :, 1, :], op=Op.subtract)
            area = glob.tile([P, G, NT], f32)
            nc.vector.tensor_tensor(out=area[:, :, :], in0=dx[:, :, :],
                                    in1=dy[:, :, :], op=Op.mult)
            wgt = glob.tile([P, G, NT], f32)
            nc.vector.tensor_tensor(out=wgt[:, :, :], in0=area[:, :, :],
                                    in1=valid[:, :, :], op=Op.mult)
            thr = glob.tile([P, G, NT], f32)
            nc.vector.tensor_scalar(out=thr[:, :, :], in0=area[:, :, :],
                                    scalar1=0.8, scalar2=None, op0=Op.mult)
            # wT[(g,t), p] = w[p, (g,t)]  (PE transpose; rows feed W_bc)
            wT_psum = psum_w.tile([P, P], f32)
            nc.tensor.transpose(out=wT_psum[:, :],
                                in_=wgt.rearrange("p g t -> p (g t)"),
                                identity=ident[:, :])
            wT_sb = glob.tile([P, P], f32)
            nc.scalar.copy(out=wT_sb[:, :], in_=wT_psum[:, :])
            nc.sync.dma_start(out=wtd[:, :], in_=wT_sb[:, :])

            cnt_sb = glob.tile([1, G], i32)

            import os as _os0
            _geff = int(_os0.environ.get("KDBG_GROUPS", str(G)))
            _reps = int(_os0.environ.get("KDBG_REPEAT", "1"))
            for _rep in range(_reps):
             for g in range(_geff):
                # ------------ broadcast tiles for group g ------------
                cbc = []
                for c in range(4):
                    t_ = bcp.tile([P, M], f32, tag=f"cbc{c}")
                    nc.sync.dma_start(
                        out=t_[:, :],
                        in_=bass.AP(tensor=bt.tensor,
                                    offset=(g * 5 + c) * M,
                                    ap=[[0, P], [1, M]]))
                    cbc.append(t_)
                x1bc, y1bc, x2bc, y2bc = cbc
                wbc = bcp.tile([P, M], f32, tag="wbc")
                for t in range(NT):
                    col = g * NT + t
                    nc.sync.dma_start(
                        out=wbc[:, t * P:(t + 1) * P],
                        in_=bass.AP(tensor=wtd.tensor, offset=col * P,
                                    ap=[[0, P], [1, P]]))

                # ------------ containment sums ------------
                S_g = sml.tile([P, NT], f32, tag="S")
                for t in range(NT):
                    x1k = neg01[:, g, 0, t:t + 1]
                    y1k = neg01[:, g, 1, t:t + 1]
                    x2k = coords_sb[:, g, 2, t:t + 1]
                    y2k = coords_sb[:, g, 3, t:t + 1]
                    s1 = work.tile([P, M], bf16, tag="s1")
                    nc.scalar.activation(out=s1[:, :], in_=x1bc[:, :],
                                         func=Act.Sign, bias=x1k, scale=1.0)
                    s2 = work.tile([P, M], bf16, tag="s2")
                    nc.scalar.activation(out=s2[:, :], in_=y1bc[:, :],
                                         func=Act.Sign, bias=y1k, scale=1.0)
                    s3 = work.tile([P, M], bf16, tag="s3")
                    nc.scalar.activation(out=s3[:, :], in_=x2bc[:, :],
                                         func=Act.Sign, bias=x2k, scale=-1.0)
                    m12 = work.tile([P, M], bf16, tag="m12")
                    nc.vector.tensor_tensor(out=m12[:, :], in0=s1[:, :],
                                            in1=s2[:, :], op=Op.min)
                    # m34 = min((y2_j <= y2_k), s3) fused in one pass
                    m34 = work.tile([P, M], bf16, tag="m34")
                    nc.vector.scalar_tensor_tensor(
                        out=m34[:, :], in0=y2bc[:, :], scalar=y2k,
                        in1=s3[:, :], op0=Op.is_le, op1=Op.min)
                    mm = work.tile([P, M], bf16, tag="mm")
                    nc.vector.tensor_tensor(out=mm[:, :], in0=m12[:, :],
                                            in1=m34[:, :], op=Op.min)
                    scr = work.tile([P, M], f32, tag="scr")
                    nc.vector.scalar_tensor_tensor(
                        out=scr[:, :], in0=mm[:, :], scalar=0.0,
                        in1=wbc[:, :], op0=Op.max, op1=Op.mult,
                        accum_out=S_g[:, t:t + 1])

                # ------------ per-group postprocessing (gpsimd) ------------
                keep = sml.tile([P, NT], f32, tag="keep")
                nc.vector.tensor_tensor(out=keep[:, :], in0=S_g[:, :],
                                        in1=thr[:, g, :], op=Op.is_le)
                nc.vector.tensor_tensor(out=keep[:, :], in0=keep[:, :],
                                        in1=valid[:, g, :], op=Op.mult)
                kp = sml.tile([P, 1], f32, tag="kp")
                nc.vector.tensor_reduce(out=kp[:, :], in_=keep[:, :],
                                        axis=mybir.AxisListType.X, op=Op.add)
                gp = psum_g.tile([P, 32], f32, tag="gp")
                pos_psum = gp[:, 0:NT]
                smt = gp[:, 16:32]
                nc.tensor.matmul(out=pos_psum[:, :], lhsT=Lmat[:, :],
                                 rhs=keep[:, :], start=True, stop=True)
                # ct = per-tile column sums [8,1]; K = total [1,1]
                nc.tensor.matmul(out=smt[:NT, 0:1], lhsT=keep[:, :],
                                 rhs=ones_col[:, :], start=True, stop=True)
                nc.tensor.matmul(out=smt[:1, 1:2], lhsT=ones_col[:, :],
                                 rhs=kp[:, :], start=True, stop=True)
                ct_sb = sml.tile([NT, 1], f32, tag="ct")
                nc.scalar.copy(out=ct_sb[:, :], in_=smt[:NT, 0:1])
                carry_psum = gp[:NT, 8:9]
                nc.tensor.matmul(out=carry_psum[:, :], lhsT=Lmat[:NT, :NT],
                                 rhs=ct_sb[:, :], start=True, stop=True)
                carry_sb = sml.tile([NT, 1], f32, tag="carrys")
                nc.scalar.copy(out=carry_sb[:, :], in_=carry_psum[:, :])
                carryT_psum = gp[:1, 8:8 + NT]
                nc.tensor.transpose(out=carryT_psum[:, :], in_=carry_sb[:, :],
                                    identity=ident[:NT, :NT])
                carryT_sb = sml.tile([1, NT + 1], f32, tag="carryTs")
                nc.scalar.copy(out=carryT_sb[:1, :NT], in_=carryT_psum[:1, :])
                nc.scalar.copy(out=carryT_sb[:1, NT:NT + 1], in_=smt[:1, 1:2])
                nc.sync.dma_start(out=ckd[g:g + 1, :], in_=carryT_sb[:1, :])
                carry_bc = sml.tile([P, NT], f32, tag="carrybc")
                nc.sync.dma_start(
                    out=carry_bc[:, :],
                    in_=bass.AP(tensor=ckd.tensor, offset=g * (NT + 1),
                                ap=[[0, P], [1, NT]]))
                K_bc = sml.tile([P, 1], f32, tag="kbc")
                nc.sync.dma_start(
                    out=K_bc[:, :],
                    in_=bass.AP(tensor=ckd.tensor, offset=g * (NT + 1) + NT,
                                ap=[[0, P], [1, 1]]))
                # cnt
                nc.vector.tensor_copy(out=cnt_sb[:1, g:g + 1],
                                      in_=smt[:1, 1:2])

                pos = sml.tile([P, NT], f32, tag="posb")
                nc.vector.tensor_tensor(out=pos[:, :], in0=pos_psum[:, :],
                                        in1=carry_bc[:, :], op=Op.add)
                # tau = keep*pos + (1-keep)*(K + iota - pos)
                b1 = sml.tile([P, NT], f32, tag="b1")
                nc.vector.tensor_scalar(out=b1[:, :], in0=iota8[:, :],
                                        scalar1=K_bc[:, :], scalar2=None,
                                        op0=Op.add)
                b2 = sml.tile([P, NT], f32, tag="b2")
                nc.vector.tensor_tensor(out=b2[:, :], in0=b1[:, :],
                                        in1=pos[:, :], op=Op.subtract)
                nk = sml.tile([P, NT], f32, tag="nk")
                nc.vector.tensor_scalar(out=nk[:, :], in0=keep[:, :],
                                        scalar1=-1.0, scalar2=1.0,
                                        op0=Op.mult, op1=Op.add)
                t1 = sml.tile([P, NT], f32, tag="t1")
                nc.vector.tensor_tensor(out=t1[:, :], in0=pos[:, :],
                                        in1=keep[:, :], op=Op.mult)
                t2 = sml.tile([P, NT], f32, tag="t2")
                nc.vector.tensor_tensor(out=t2[:, :], in0=b2[:, :],
                                        in1=nk[:, :], op=Op.mult)
                tau = sml.tile([P, NT], f32, tag="tau")
                nc.vector.tensor_tensor(out=tau[:, :], in0=t1[:, :],
                                        in1=t2[:, :], op=Op.add)
                import os as _os2
                if _os2.environ.get("KDBG_DUMP_TAU"):
                    nc.sync.dma_start(out=dbg[:, g * NT:(g + 1) * NT],
                                      in_=tau[:, :])
                offi = sml.tile([P, NT], i32, tag="offi")
                nc.vector.tensor_scalar(out=offi[:, :], in0=tau[:, :],
                                        scalar1=float(g * M), scalar2=None,
                                        op0=Op.add)
                # mask = keep or (K == 0)
                kz = sml.tile([P, 1], f32, tag="kz")
                nc.vector.tensor_scalar(out=kz[:, :], in0=K_bc[:, :],
                                        scalar1=0.0, scalar2=None,
                                        op0=Op.is_equal)
                mask = sml.tile([P, NT], f32, tag="mask")
                nc.vector.tensor_tensor(out=mask[:, :], in0=keep[:, :],
                                        in1=fap(kz, 0, [[0, NT]]),
                                        op=Op.max)
                # masked rows in (t c) layout: mb[p, t, c]
                mb = sml.tile([P, NT, 5], f32, tag="mb")
                nc.vector.tensor_tensor(
                    out=mb[:, :, :],
                    in0=fap(coords_sb, g * 5 * NT, [[1, NT], [NT, 5]]),
                    in1=fap(mask, 0, [[1, NT], [0, 5]]), op=Op.mult)
                # scatter rows t-by-t
                import os as _os
                _sg = int(_os.environ.get("KDBG_SCATTER_GROUPS", "16"))
                if _os.environ.get("KDBG_NO_SCATTER") or g >= _sg:
                    # identity placement (row = g*1024 + t*128 + p), no gather
                    nc.sync.dma_start(
                        out=bass.AP(tensor=out_boxes[:, :].tensor,
                                    offset=g * M * 5,
                                    ap=[[5, P], [5 * P, NT], [1, 5]]),
                        in_=mb[:, :, :])
                else:
                    for t in range(NT):
                        nc.gpsimd.indirect_dma_start(
                            out=out_boxes[:, :],
                            out_offset=bass.IndirectOffsetOnAxis(
                                ap=offi[:, t:t + 1], axis=0),
                            in_=mb[:, t, :], in_offset=None)

            nc.sync.dma_start(out=out_cnt[:1, :], in_=cnt_sb[:1, :])

    _CACHE["nc"] = nc
    return nc


def kernel(box_prompts, num_boxes):
    from concourse.bass_utils import run_bass_kernel_spmd

    nc = _build_program()
    T, C, Mm, F = box_prompts.shape
    flat_b = np.ascontiguousarray(box_prompts.reshape(T * C, Mm, F))
    flat_n = num_boxes.reshape(T * C)

    in_maps = []
    for c in range(N_CORES):
        sl = slice(c * G, (c + 1) * G)
        in_maps.append({
            "boxesT": np.ascontiguousarray(
                flat_b[sl].transpose(0, 2, 1)).astype(np.float32),
            "nf": flat_n[sl].astype(np.float32).reshape(1, G),
        })

    res = run_bass_kernel_spmd(nc, in_maps, core_ids=list(range(N_CORES)))
    outs = res.results
    boxes = np.concatenate(
        [outs[c]["out_boxes"].reshape(G, Mm, F) for c in range(N_CORES)],
        axis=0).reshape(T, C, Mm, F).astype(box_prompts.dtype)
    cnt = np.concatenate(
        [outs[c]["out_cnt"].reshape(G) for c in range(N_CORES)],
        axis=0).reshape(T, C).astype(num_boxes.dtype)
    return boxes, cnt
